# revision 10
# baseline (speedup 1.0000x reference)
"""KoLeo-loss kernel, 8 NeuronCores — hybrid AllGather + HBM streaming.

Two independent transports feed the key groups:
  - groups 0..3: 2 chunked AllGathers of the packed-transposed own rows
    (sub-chunks q=0,1 of every rank; 256KB/rank -> 2MB each).
  - groups 4..7: streamed from HBM (sub-chunks q=2,3 of every rank, 16
    chunks of 128 rows), normalized with rinv fetched via a tiny (4KB/rank)
    rinv-AllGather — so streamed chunks skip Square/rsqrt entirely.

Uniform layout rule both paths obey: group g = 2q+h holds sub-chunk q of
ranks 4h..4h+3, rank band (r%4)*128. The diagonal of core r for query
sub-chunk mc lives in group 2*mc+(r>=4) at band (r%4)*128+p: injected by a
host-fed band pattern (rhs) x half-selector (lhsT) bf16 matmul.

Math: fp8e4 keys scaled 32/||x||, t-paired DoubleRow Gram = 1024*cos,
row max M -> loss_i = -0.5*ln(2 - M/512); host sums 8 partials.
"""

import sys
from contextlib import ExitStack

import numpy as np

sys.path.insert(0, "/opt/trn_rl_repo")

import concourse.mybir as mybir
import concourse.tile as tile
from concourse import bacc, bass_utils

F32 = mybir.dt.float32
BF16 = mybir.dt.bfloat16
F8 = mybir.dt.float8e4
AF = mybir.ActivationFunctionType
DR = mybir.MatmulPerfMode.DoubleRow

B, V, D = 4096, 2, 1024
NCORES = 8
MB = B // NCORES          # 512 own rows per core
NQ = MB // 128            # 4 own sub-chunks
NG = 8
T = 2
EPS = 1e-8
MASKV = -8192.0
AGQ = (0,)                # sub-chunks delivered by AllGather
STQ = (1, 2, 3)           # sub-chunks streamed from HBM


def build():
    nc = bacc.Bacc("TRN2", debug=False, num_devices=NCORES)
    x_d = nc.dram_tensor("x", [B, V, D], F32, kind="ExternalInput").ap()
    xq_d = nc.dram_tensor("xq", [MB, V, D], F32, kind="ExternalInput").ap()
    band_d = nc.dram_tensor("bandpat", [128, 512], F32, kind="ExternalInput").ap()
    hsel_d = nc.dram_tensor("hsel", [128, 2], F32, kind="ExternalInput").ap()
    out_d = nc.dram_tensor("out", [1, 1], F32, kind="ExternalOutput").ap()

    with ExitStack() as ctx:
        tc = ctx.enter_context(tile.TileContext(nc))
        const = ctx.enter_context(tc.tile_pool(name="const", bufs=1))
        xpool = ctx.enter_context(tc.tile_pool(name="xpool", bufs=8))
        ypool = ctx.enter_context(tc.tile_pool(name="ypool", bufs=3))
        sqpool = ctx.enter_context(tc.tile_pool(name="sqpool", bufs=2))
        sspool = ctx.enter_context(tc.tile_pool(name="sspool", bufs=2))
        accp = ctx.enter_context(tc.tile_pool(name="accp", bufs=3, space="PSUM"))
        trp = ctx.enter_context(tc.tile_pool(name="trp", bufs=3, space="PSUM"))
        smallp = ctx.enter_context(tc.tile_pool(name="smallp", bufs=2, space="PSUM"))
        dram = ctx.enter_context(tc.tile_pool(name="dram", bufs=1, space="DRAM"))

        # ---- constants ----
        identF = const.tile([128, 128], F32, name="identF")
        nc.gpsimd.memset(identF[:], 0.0)
        nc.gpsimd.affine_select(
            out=identF[:], in_=identF[:], compare_op=mybir.AluOpType.not_equal,
            fill=1.0, base=0, pattern=[[-1, 128]], channel_multiplier=1)
        identB = const.tile([128, 128], BF16, name="identB")
        nc.gpsimd.memset(identB[:], 0.0)
        nc.gpsimd.affine_select(
            out=identB[:], in_=identB[:], compare_op=mybir.AluOpType.not_equal,
            fill=1.0, base=0, pattern=[[-1, 128]], channel_multiplier=1)
        ones = const.tile([128, 1], F32, name="ones")
        nc.vector.memset(ones[:], 1.0)
        epsb = const.tile([128, 1], F32, name="epsb")
        nc.gpsimd.memset(epsb[:], EPS)

        bandF = const.tile([128, 512], F32, name="bandF")
        nc.sync.dma_start(bandF[:], band_d)
        bandB = const.tile([128, 512], BF16, name="bandB")
        nc.vector.tensor_copy(bandB[:], bandF[:])
        hs = const.tile([128, 2], F32, name="hs")
        nc.sync.dma_start(hs[:], hsel_d)
        hselI = const.tile([128, 2, 128], BF16, name="hselI")
        for h in range(2):
            nc.gpsimd.tensor_scalar_mul(hselI[:, h, :], identB[:], hs[:, h : h + 1])

        wrm = trp.tile([128, 128], F32, tag="tp", name="wrm")
        for _ in range(20):
            nc.tensor.transpose(wrm[:], identF[:], identF[:])

        # ---- persistent buffers ----
        QT = const.tile([128, V, T, MB], F32, name="QT")
        YTg = [const.tile([128, V, T, 512], F32, name=f"YT{g}") for g in range(NG)]
        mxs = const.tile([128, NG, V * NQ], F32, name="mxs")
        rsq = const.tile([128, NQ, V], F32, name="rsq")       # own 32/||x||

        agin = [dram.tile([128, V, T, 128], F32, name=f"agin{q}") for q in AGQ]
        agout = [dram.tile([NCORES, 128, V, T, 128], F32, name=f"agout{q}")
                 for q in AGQ]

        # ---- own chunks: full normalize -> QT; rs kept in rsq ----
        for qc in range(NQ):
            xt = xpool.tile([128, V, D], F32, tag="xraw", name="xraw")
            nc.sync.dma_start(xt[:], xq_d[128 * qc : 128 * (qc + 1)])
            ss = sspool.tile([128, V], F32, tag="ss", name="ss")
            sq = sqpool.tile([128, D], BF16, tag="sq", name="sq")
            for v in range(V):
                nc.scalar.activation(
                    sq[:], xt[:, v, :], AF.Square, accum_out=ss[:, v : v + 1])
            rec = sspool.tile([128, V], F32, tag="rec", name="rec")
            nc.vector.tensor_scalar_add(rec[:], ss[:], EPS)
            nc.vector.reciprocal(rec[:], rec[:])
            nc.scalar.activation(rsq[:, qc, :], rec[:], AF.Sqrt, scale=1024.0)
            rsv = sspool.tile([128, V], F32, tag="rsv", name="rsv")
            nc.vector.tensor_copy(rsv[:], rsq[:, qc, :])

            ypk = ypool.tile([128, V, T, 128], F32, tag="ypk", name="ypk")
            yp8 = ypk.bitcast(F8)
            nc.vector.tensor_scalar_mul(
                yp8[:, 0].rearrange("p t k -> p (t k)"), xt[:, 0, :], rsv[:, 0:1])
            nc.vector.tensor_scalar_mul(
                yp8[:, 1].rearrange("p t k -> p (t k)"), xt[:, 1, :], rsv[:, 1:2])
            for v in range(V):
                for t in range(T):
                    tp = trp.tile([128, 128], F32, tag="tp", name="tp")
                    nc.tensor.transpose(tp[:], ypk[:, v, t], identF[:])
                    nc.vector.tensor_copy(
                        QT[:, v, t, 128 * qc : 128 * (qc + 1)], tp[:])
            if qc in AGQ:
                nc.gpsimd.dma_start(
                    agin[qc][:], QT[:, :, :, 128 * qc : 128 * (qc + 1)])
                nc.gpsimd.collective_compute(
                    "AllGather", mybir.AluOpType.bypass,
                    replica_groups=[list(range(NCORES))],
                    ins=[agin[qc].opt()], outs=[agout[qc].opt()])
                for r in range(NCORES):
                    nc.gpsimd.dma_start(
                        YTg[2 * qc + r // 4][:, :, :,
                                             128 * (r % 4) : 128 * (r % 4) + 128],
                        agout[qc][r])

        # ---- streamed chunks (sub-chunks q=2,3 of every rank), local norm ----
        for q in STQ:
            for rr in range(NCORES):
                row0 = rr * MB + q * 128
                xt = xpool.tile([128, V, D], F32, tag="xraw", name="xraw")
                nc.sync.dma_start(xt[:], x_d[row0 : row0 + 128])
                ss = sspool.tile([128, V], F32, tag="ss", name="ss")
                sq = sqpool.tile([128, D], BF16, tag="sq", name="sq")
                for v in range(V):
                    nc.scalar.activation(
                        sq[:], xt[:, v, :], AF.Square,
                        accum_out=ss[:, v : v + 1])
                rec = sspool.tile([128, V], F32, tag="rec", name="rec")
                nc.vector.tensor_scalar_add(rec[:], ss[:], EPS)
                nc.vector.reciprocal(rec[:], rec[:])
                rs = sspool.tile([128, V], F32, tag="rs", name="rs")
                nc.scalar.activation(rs[:], rec[:], AF.Sqrt, scale=1024.0)
                rsv = sspool.tile([128, V], F32, tag="rsv", name="rsv")
                nc.vector.tensor_copy(rsv[:], rs[:])
                ypk = ypool.tile([128, V, T, 128], F32, tag="ypk", name="ypk")
                yp8 = ypk.bitcast(F8)
                nc.scalar.activation(
                    yp8[:, 0].rearrange("p t k -> p (t k)"), xt[:, 0, :],
                    AF.Copy, scale=rs[:, 0:1])
                nc.vector.tensor_scalar_mul(
                    yp8[:, 1].rearrange("p t k -> p (t k)"), xt[:, 1, :],
                    rsv[:, 1:2])
                g = 2 * q + rr // 4
                for v in range(V):
                    for t in range(T):
                        tp = trp.tile([128, 128], F32, tag="tp", name="tp")
                        nc.tensor.transpose(tp[:], ypk[:, v, t], identF[:])
                        nc.vector.tensor_copy(
                            YTg[g][:, v, t,
                                   128 * (rr % 4) : 128 * (rr % 4) + 128], tp[:])

        # ---- per-group Gram rows + row max ----
        # streamed groups first in PE program order (their data arrives
        # progressively), AG groups interleaved after their DMAs land.
        Q8r = QT.bitcast(F8)[:].rearrange("p v t (m b) -> p v b t m", b=4)
        for g in (2, 3, 4, 5, 0, 1, 6, 7):
            Y8r = YTg[g].bitcast(F8)[:].rearrange("p v t (k b) -> p v b t k", b=4)
            q_of_g, h_of_g = g // 2, g % 2
            for v in range(V):
                for mc in range(NQ):
                    has_mask = mc == q_of_g
                    acc = accp.tile([128, 512], F32, tag="acc", name="acc")
                    for b in range(4):
                        nc.tensor.matmul(
                            acc[:],
                            Q8r[:, v, b, :, 128 * mc : 128 * (mc + 1)],
                            Y8r[:, v, b, :, :],
                            start=(b == 0), stop=(b == 3 and not has_mask),
                            perf_mode=DR)
                    if has_mask:
                        nc.tensor.matmul(
                            acc[:], hselI[:, h_of_g], bandB[:],
                            start=False, stop=True, skip_group_check=True)
                    nc.vector.reduce_max(
                        mxs[:, g, v * NQ + mc : v * NQ + mc + 1], acc[:],
                        axis=mybir.AxisListType.X)

        # ---- finale ----
        fm = const.tile([128, V * NQ], F32, name="fm")
        nc.vector.reduce_max(
            fm[:], mxs.rearrange("p g c -> p c g"), axis=mybir.AxisListType.X)
        tt = const.tile([128, V * NQ], F32, name="tt")
        nc.vector.tensor_scalar(
            tt[:], fm[:], -1.0 / 512.0, 2.0, mybir.AluOpType.mult,
            mybir.AluOpType.add)
        lg = const.tile([128, V * NQ], F32, name="lg")
        nc.scalar.activation(lg[:], tt[:], AF.Ln, bias=epsb[:])
        ps2 = smallp.tile([1, V * NQ], F32, tag="sps", name="ps2")
        nc.tensor.matmul(ps2[:], ones[:], lg[:], start=True, stop=True)
        tot = const.tile([1, 1], F32, name="tot")
        nc.vector.reduce_sum(tot[:], ps2[:], axis=mybir.AxisListType.X)
        tots = const.tile([1, 1], F32, name="tots")
        nc.vector.tensor_scalar_mul(tots[:], tot[:], -0.5 / B)
        nc.sync.dma_start(out_d, tots[:])

    nc.compile()
    return nc


_CACHED = {}


def _run(x, trace=False):
    x = np.ascontiguousarray(np.asarray(x, dtype=np.float32))
    assert x.shape == (B, V, D), x.shape
    if "nc" not in _CACHED:
        _CACHED["nc"] = build()
    nc = _CACHED["nc"]
    in_maps = []
    for r in range(NCORES):
        band = np.zeros((128, 512), np.float32)
        col0 = (r % 4) * 128
        band[np.arange(128), col0 + np.arange(128)] = MASKV
        hsel = np.zeros((1, 2), np.float32)
        hsel[0, r // 4] = 1.0
        in_maps.append({
            "x": x,
            "xq": np.ascontiguousarray(x[MB * r : MB * (r + 1)]),
            "bandpat": band,
            "hsel": np.broadcast_to(hsel, (128, 2)).copy(),
        })
    res = bass_utils.run_bass_kernel_spmd(
        nc, in_maps, core_ids=list(range(NCORES)), trace=trace)
    partials = [np.float32(res.results[r]["out"][0, 0]) for r in range(NCORES)]
    total = np.float32(np.sum(np.array(partials, dtype=np.float32)))
    return total, res


def kernel(student_global_cls_tokens):
    total, _ = _run(student_global_cls_tokens, trace=False)
    return np.asarray(total, dtype=np.float32)


# revision 11
# speedup vs baseline: 1.1153x; 1.1153x over previous
"""KoLeo-loss kernel, 8 NeuronCores — hybrid AllGather + HBM streaming.

Two independent transports feed the key groups:
  - groups 0..3: 2 chunked AllGathers of the packed-transposed own rows
    (sub-chunks q=0,1 of every rank; 256KB/rank -> 2MB each).
  - groups 4..7: streamed from HBM (sub-chunks q=2,3 of every rank, 16
    chunks of 128 rows), normalized with rinv fetched via a tiny (4KB/rank)
    rinv-AllGather — so streamed chunks skip Square/rsqrt entirely.

Uniform layout rule both paths obey: group g = 2q+h holds sub-chunk q of
ranks 4h..4h+3, rank band (r%4)*128. The diagonal of core r for query
sub-chunk mc lives in group 2*mc+(r>=4) at band (r%4)*128+p: injected by a
host-fed band pattern (rhs) x half-selector (lhsT) bf16 matmul.

Math: fp8e4 keys scaled 32/||x||, t-paired DoubleRow Gram = 1024*cos,
row max M -> loss_i = -0.5*ln(2 - M/512); host sums 8 partials.
"""

import sys
from contextlib import ExitStack

import numpy as np

sys.path.insert(0, "/opt/trn_rl_repo")

import concourse.mybir as mybir
import concourse.tile as tile
from concourse import bacc, bass_utils

F32 = mybir.dt.float32
BF16 = mybir.dt.bfloat16
F8 = mybir.dt.float8e4
AF = mybir.ActivationFunctionType
DR = mybir.MatmulPerfMode.DoubleRow

B, V, D = 4096, 2, 1024
NCORES = 8
MB = B // NCORES          # 512 own rows per core
NQ = MB // 128            # 4 own sub-chunks
NG = 8
T = 2
EPS = 1e-8
MASKV = -8192.0
AGQ = (0, 1)              # sub-chunks delivered by AllGather
STQ = (2, 3)              # sub-chunks streamed from HBM


def build():
    nc = bacc.Bacc("TRN2", debug=False, num_devices=NCORES)
    x_d = nc.dram_tensor("x", [B, V, D], F32, kind="ExternalInput").ap()
    xq_d = nc.dram_tensor("xq", [MB, V, D], F32, kind="ExternalInput").ap()
    band_d = nc.dram_tensor("bandpat", [128, 512], F32, kind="ExternalInput").ap()
    hsel_d = nc.dram_tensor("hsel", [128, 2], F32, kind="ExternalInput").ap()
    out_d = nc.dram_tensor("out", [1, 1], F32, kind="ExternalOutput").ap()

    with ExitStack() as ctx:
        tc = ctx.enter_context(tile.TileContext(nc))
        const = ctx.enter_context(tc.tile_pool(name="const", bufs=1))
        xpool = ctx.enter_context(tc.tile_pool(name="xpool", bufs=6))
        ypool = ctx.enter_context(tc.tile_pool(name="ypool", bufs=3))
        sqpool = ctx.enter_context(tc.tile_pool(name="sqpool", bufs=2))
        sspool = ctx.enter_context(tc.tile_pool(name="sspool", bufs=2))
        accp = ctx.enter_context(tc.tile_pool(name="accp", bufs=3, space="PSUM"))
        trp = ctx.enter_context(tc.tile_pool(name="trp", bufs=3, space="PSUM"))
        smallp = ctx.enter_context(tc.tile_pool(name="smallp", bufs=2, space="PSUM"))
        dram = ctx.enter_context(tc.tile_pool(name="dram", bufs=1, space="DRAM"))

        # ---- constants ----
        identF = const.tile([128, 128], F32, name="identF")
        nc.gpsimd.memset(identF[:], 0.0)
        nc.gpsimd.affine_select(
            out=identF[:], in_=identF[:], compare_op=mybir.AluOpType.not_equal,
            fill=1.0, base=0, pattern=[[-1, 128]], channel_multiplier=1)
        identB = const.tile([128, 128], BF16, name="identB")
        nc.gpsimd.memset(identB[:], 0.0)
        nc.gpsimd.affine_select(
            out=identB[:], in_=identB[:], compare_op=mybir.AluOpType.not_equal,
            fill=1.0, base=0, pattern=[[-1, 128]], channel_multiplier=1)
        ones = const.tile([128, 1], F32, name="ones")
        nc.vector.memset(ones[:], 1.0)
        epsb = const.tile([128, 1], F32, name="epsb")
        nc.gpsimd.memset(epsb[:], EPS)

        bandF = const.tile([128, 512], F32, name="bandF")
        nc.sync.dma_start(bandF[:], band_d)
        bandB = const.tile([128, 512], BF16, name="bandB")
        nc.vector.tensor_copy(bandB[:], bandF[:])
        hs = const.tile([128, 2], F32, name="hs")
        nc.sync.dma_start(hs[:], hsel_d)
        hselI = const.tile([128, 2, 128], BF16, name="hselI")
        for h in range(2):
            nc.gpsimd.tensor_scalar_mul(hselI[:, h, :], identB[:], hs[:, h : h + 1])

        wrm = trp.tile([128, 128], F32, tag="tp", name="wrm")
        for _ in range(20):
            nc.tensor.transpose(wrm[:], identF[:], identF[:])

        # ---- persistent buffers ----
        QT = const.tile([128, V, T, MB], F32, name="QT")
        YTg = [const.tile([128, V, T, 512], F32, name=f"YT{g}") for g in range(NG)]
        mxs = const.tile([128, NG, V * NQ], F32, name="mxs")
        rsq = const.tile([128, NQ, V], F32, name="rsq")       # own 32/||x||

        agin = [dram.tile([128, V, T, 128], F32, name=f"agin{q}") for q in AGQ]
        agout = [dram.tile([NCORES, 128, V, T, 128], F32, name=f"agout{q}")
                 for q in AGQ]

        # ---- own chunks: full normalize -> QT; rs kept in rsq ----
        for qc in range(NQ):
            xt = xpool.tile([128, V, D], F32, tag="xraw", name="xraw")
            nc.sync.dma_start(xt[:], xq_d[128 * qc : 128 * (qc + 1)])
            ss = sspool.tile([128, V], F32, tag="ss", name="ss")
            sq = sqpool.tile([128, D], BF16, tag="sq", name="sq")
            for v in range(V):
                nc.scalar.activation(
                    sq[:], xt[:, v, :], AF.Square, accum_out=ss[:, v : v + 1])
            rec = sspool.tile([128, V], F32, tag="rec", name="rec")
            nc.vector.tensor_scalar_add(rec[:], ss[:], EPS)
            nc.vector.reciprocal(rec[:], rec[:])
            nc.scalar.activation(rsq[:, qc, :], rec[:], AF.Sqrt, scale=1024.0)
            rsv = sspool.tile([128, V], F32, tag="rsv", name="rsv")
            nc.vector.tensor_copy(rsv[:], rsq[:, qc, :])

            ypk = ypool.tile([128, V, T, 128], F32, tag="ypk", name="ypk")
            yp8 = ypk.bitcast(F8)
            nc.scalar.activation(
                yp8[:, 0].rearrange("p t k -> p (t k)"), xt[:, 0, :], AF.Copy,
                scale=rsq[:, qc, 0:1])
            nc.vector.tensor_scalar_mul(
                yp8[:, 1].rearrange("p t k -> p (t k)"), xt[:, 1, :], rsv[:, 1:2])
            for v in range(V):
                for t in range(T):
                    tp = trp.tile([128, 128], F32, tag="tp", name="tp")
                    nc.tensor.transpose(tp[:], ypk[:, v, t], identF[:])
                    nc.vector.tensor_copy(
                        QT[:, v, t, 128 * qc : 128 * (qc + 1)], tp[:])
            if qc in AGQ:
                nc.gpsimd.dma_start(
                    agin[qc][:], QT[:, :, :, 128 * qc : 128 * (qc + 1)])
                nc.gpsimd.collective_compute(
                    "AllGather", mybir.AluOpType.bypass,
                    replica_groups=[list(range(NCORES))],
                    ins=[agin[qc].opt()], outs=[agout[qc].opt()])
                for r in range(NCORES):
                    nc.gpsimd.dma_start(
                        YTg[2 * qc + r // 4][:, :, :,
                                             128 * (r % 4) : 128 * (r % 4) + 128],
                        agout[qc][r])

        # ---- streamed chunks (sub-chunks q=2,3 of every rank), local norm ----
        for q in STQ:
            for rr in range(NCORES):
                row0 = rr * MB + q * 128
                xt = xpool.tile([128, V, D], F32, tag="xraw", name="xraw")
                nc.sync.dma_start(xt[:], x_d[row0 : row0 + 128])
                ss = sspool.tile([128, V], F32, tag="ss", name="ss")
                sq = sqpool.tile([128, D], BF16, tag="sq", name="sq")
                for v in range(V):
                    nc.scalar.activation(
                        sq[:], xt[:, v, :], AF.Square,
                        accum_out=ss[:, v : v + 1])
                rec = sspool.tile([128, V], F32, tag="rec", name="rec")
                nc.vector.tensor_scalar_add(rec[:], ss[:], EPS)
                nc.vector.reciprocal(rec[:], rec[:])
                rs = sspool.tile([128, V], F32, tag="rs", name="rs")
                nc.scalar.activation(rs[:], rec[:], AF.Sqrt, scale=1024.0)
                rsv = sspool.tile([128, V], F32, tag="rsv", name="rsv")
                nc.vector.tensor_copy(rsv[:], rs[:])
                ypk = ypool.tile([128, V, T, 128], F32, tag="ypk", name="ypk")
                yp8 = ypk.bitcast(F8)
                nc.scalar.activation(
                    yp8[:, 0].rearrange("p t k -> p (t k)"), xt[:, 0, :],
                    AF.Copy, scale=rs[:, 0:1])
                nc.vector.tensor_scalar_mul(
                    yp8[:, 1].rearrange("p t k -> p (t k)"), xt[:, 1, :],
                    rsv[:, 1:2])
                g = 2 * q + rr // 4
                for v in range(V):
                    for t in range(T):
                        tp = trp.tile([128, 128], F32, tag="tp", name="tp")
                        nc.tensor.transpose(tp[:], ypk[:, v, t], identF[:])
                        nc.vector.tensor_copy(
                            YTg[g][:, v, t,
                                   128 * (rr % 4) : 128 * (rr % 4) + 128], tp[:])

        # ---- per-group Gram rows + row max ----
        # streamed groups first in PE program order (their data arrives
        # progressively), AG groups interleaved after their DMAs land.
        Q8r = QT.bitcast(F8)[:].rearrange("p v t (m b) -> p v b t m", b=4)
        for g in (4, 5, 0, 1, 6, 7, 2, 3):
            Y8r = YTg[g].bitcast(F8)[:].rearrange("p v t (k b) -> p v b t k", b=4)
            q_of_g, h_of_g = g // 2, g % 2
            for v in range(V):
                for mc in range(NQ):
                    has_mask = mc == q_of_g
                    acc = accp.tile([128, 512], F32, tag="acc", name="acc")
                    for b in range(4):
                        nc.tensor.matmul(
                            acc[:],
                            Q8r[:, v, b, :, 128 * mc : 128 * (mc + 1)],
                            Y8r[:, v, b, :, :],
                            start=(b == 0), stop=(b == 3 and not has_mask),
                            perf_mode=DR)
                    if has_mask:
                        nc.tensor.matmul(
                            acc[:], hselI[:, h_of_g], bandB[:],
                            start=False, stop=True, skip_group_check=True)
                    nc.vector.reduce_max(
                        mxs[:, g, v * NQ + mc : v * NQ + mc + 1], acc[:],
                        axis=mybir.AxisListType.X)

        # ---- finale ----
        fm = const.tile([128, V * NQ], F32, name="fm")
        nc.vector.reduce_max(
            fm[:], mxs.rearrange("p g c -> p c g"), axis=mybir.AxisListType.X)
        tt = const.tile([128, V * NQ], F32, name="tt")
        nc.vector.tensor_scalar(
            tt[:], fm[:], -1.0 / 512.0, 2.0, mybir.AluOpType.mult,
            mybir.AluOpType.add)
        lg = const.tile([128, V * NQ], F32, name="lg")
        nc.scalar.activation(lg[:], tt[:], AF.Ln, bias=epsb[:])
        ps2 = smallp.tile([1, V * NQ], F32, tag="sps", name="ps2")
        nc.tensor.matmul(ps2[:], ones[:], lg[:], start=True, stop=True)
        tot = const.tile([1, 1], F32, name="tot")
        nc.vector.reduce_sum(tot[:], ps2[:], axis=mybir.AxisListType.X)
        tots = const.tile([1, 1], F32, name="tots")
        nc.vector.tensor_scalar_mul(tots[:], tot[:], -0.5 / B)
        nc.sync.dma_start(out_d, tots[:])

    nc.compile()
    return nc


_CACHED = {}


def _run(x, trace=False):
    x = np.ascontiguousarray(np.asarray(x, dtype=np.float32))
    assert x.shape == (B, V, D), x.shape
    if "nc" not in _CACHED:
        _CACHED["nc"] = build()
    nc = _CACHED["nc"]
    in_maps = []
    for r in range(NCORES):
        band = np.zeros((128, 512), np.float32)
        col0 = (r % 4) * 128
        band[np.arange(128), col0 + np.arange(128)] = MASKV
        hsel = np.zeros((1, 2), np.float32)
        hsel[0, r // 4] = 1.0
        in_maps.append({
            "x": x,
            "xq": np.ascontiguousarray(x[MB * r : MB * (r + 1)]),
            "bandpat": band,
            "hsel": np.broadcast_to(hsel, (128, 2)).copy(),
        })
    res = bass_utils.run_bass_kernel_spmd(
        nc, in_maps, core_ids=list(range(NCORES)), trace=trace)
    partials = [np.float32(res.results[r]["out"][0, 0]) for r in range(NCORES)]
    total = np.float32(np.sum(np.array(partials, dtype=np.float32)))
    return total, res


def kernel(student_global_cls_tokens):
    total, _ = _run(student_global_cls_tokens, trace=False)
    return np.asarray(total, dtype=np.float32)
